# revision 3
# baseline (speedup 1.0000x reference)
"""GNN message-passing kernel for Trainium2 (8 NeuronCores, SPMD).

Reference computation (B=1, N=20000, K=32, D=128, DEPTH=3):
    h0 = graph
    for t in 1..2:
        g[n]  = mean_k h_{t-1}[adj[k, n]]        (neighbor gather + mean)
        h_t   = relu(g @ W[t] + b[t])
    out = stack([h0, h1, h2])                     # [1, 3, N, D]

(matmul and mean commute, so we gather+sum first and multiply once per
 node; the 1/K is folded into W host-side.)

Distribution: nodes sharded 2500/core (padded to 2560 = 20 chunks of 128).
Gather sources are kept TRANSPOSED (feature-major, [D=128 partitions,
nodes free]) in SBUF so the neighbor gather runs on the GPSIMD compute
path (`ap_gather`, out = in[:, idxs]) instead of per-row DMA descriptors
— the baseline's dma_gather spent ~1.5us/descriptor on random 256B HBM
reads; ap_gather streams 16 partitions x 4B per index through the Q7
engine FIFOs entirely inside SBUF. ap_gather has a ~90us fixed cost per
call, so chunks are processed two at a time (8192 indices per call).

Per pair of chunks (256 nodes, idx order i = n_off*32 + k):
    GT[d, i]   = srcT[d, idx[i]]                  (ap_gather, fp32)
    gsum[d, n] = sum_k GT[d, 32n+k]               (DVE tensor_reduce)
    gt = bf16(gsum)                               (ACT copy)
    phhT[e, n] = sum_d W'[d,e] gt[d,n]            (PE matmul, W'=W/K)
    hT[:, n]   = relu(phhT + b)                   (DVE add + max)
Layer 1 extra: hT -> DRAM -> AllGather(fp32) -> 8 block loads back into
the (reused) source tile -> layer 2 gathers from it.
Outputs are feature-major [128, NSP]; the host transposes back.
"""

import numpy as np

import concourse.bacc as bacc
import concourse.mybir as mybir
import concourse.tile as tile
from concourse.bass_utils import run_bass_kernel_spmd

# problem constants (hardcoded per harness contract)
N, K, D = 20000, 32, 128
NCORES = 8
NS = N // NCORES  # 2500 real nodes per core
CHUNK = 128
NCH = (NS + CHUNK - 1) // CHUNK  # 20 chunks
NSP = NCH * CHUNK  # 2560 padded nodes per core
NGTOT = NCORES * NSP  # 20480 nodes in the all-gathered layer-2 source
NIDX = CHUNK * K  # 4096 gather indices per chunk
CPC = 2  # chunks per ap_gather call
NCALL = NCH // CPC  # 10 calls per layer
GIDX = NIDX * CPC  # 8192 indices per call
GNOD = CHUNK * CPC  # 256 nodes per call
IDXC = GIDX // 16  # 512 idx columns in SBUF layout per call

BF16 = mybir.dt.bfloat16
NP_BF16 = mybir.dt.np(BF16)

_COMPILED = {}


def _build(repeat: int = 1):
    f32 = mybir.dt.float32
    i16 = mybir.dt.int16
    nc = bacc.Bacc(
        "TRN2",
        target_bir_lowering=False,
        debug=False,
        enable_asserts=False,
        num_devices=NCORES,
    )
    hsrc0T = nc.dram_tensor("hsrc0T", [D, N], f32, kind="ExternalInput")
    idx1 = nc.dram_tensor("idx1", [128, NCALL, IDXC], i16, kind="ExternalInput")
    idx2 = nc.dram_tensor("idx2", [128, NCALL, IDXC], i16, kind="ExternalInput")
    wmat = nc.dram_tensor("wmat", [128, 2, D], BF16, kind="ExternalInput")
    brep = nc.dram_tensor("brep", [128, 2, GNOD], f32, kind="ExternalInput")
    out1 = nc.dram_tensor("out1T", [D, NSP], f32, kind="ExternalOutput")
    out2 = nc.dram_tensor("out2T", [D, NSP], f32, kind="ExternalOutput")

    with tile.TileContext(nc) as tc:
        with (
            tc.tile_pool(name="const", bufs=1) as const,
            tc.tile_pool(name="src", bufs=1) as srcp,
            tc.tile_pool(name="g", bufs=2) as gp,
            tc.tile_pool(name="gs", bufs=2) as gsp,
            tc.tile_pool(name="gt", bufs=2) as gtp,
            tc.tile_pool(name="ph", bufs=2, space="PSUM") as ph,
            tc.tile_pool(name="h", bufs=1) as hp,
            tc.tile_pool(name="dram", bufs=repeat, space="DRAM") as dram,
        ):
            idx_sb = const.tile([128, 2, NCALL, IDXC], i16)
            nc.sync.dma_start(idx_sb[:, 0, :, :], idx1[:])
            nc.sync.dma_start(idx_sb[:, 1, :, :], idx2[:])
            w_sb = const.tile([128, 2, D], BF16)
            nc.sync.dma_start(w_sb[:], wmat[:])
            b_sb = const.tile([128, 2, GNOD], f32)
            nc.sync.dma_start(b_sb[:], brep[:])

            src = srcp.tile([128, NGTOT], f32)
            h1T = hp.tile([128, NSP], f32)
            h2T = hp.tile([128, NSP], f32)

            def layer(src_ap, ne, lidx, hT):
                for m in range(NCALL):
                    GT = gp.tile([128, GIDX], f32, tag="GT")
                    nc.gpsimd.ap_gather(
                        GT[:],
                        src_ap,
                        idx_sb[:, lidx, m, :],
                        channels=128,
                        num_elems=ne,
                        d=1,
                        num_idxs=GIDX,
                    )
                    gsum = gsp.tile([128, GNOD], f32, tag="gsum")
                    nc.vector.tensor_reduce(
                        gsum[:],
                        GT[:].rearrange("p (n k) -> p n k", k=K),
                        axis=mybir.AxisListType.X,
                        op=mybir.AluOpType.add,
                    )
                    gt = gtp.tile([128, GNOD], BF16, tag="gt")
                    nc.scalar.copy(gt[:], gsum[:])
                    phh = ph.tile([128, GNOD], f32, tag="phh")
                    nc.tensor.matmul(
                        phh[:],
                        lhsT=w_sb[:, lidx, :],
                        rhs=gt[:],
                        start=True,
                        stop=True,
                    )
                    hs = hT[:, m * GNOD : (m + 1) * GNOD]
                    nc.vector.tensor_add(hs, phh[:], b_sb[:, lidx, :])
                    nc.vector.tensor_scalar_max(hs, hs, 0.0)

            for _ in range(repeat):
                nc.sync.dma_start(src[:, :N], hsrc0T[:])
                layer(src[:, :N], N, 0, h1T)
                ag_in = dram.tile([D, NSP], f32, tag="ag_in")
                ag_out = dram.tile(
                    [NCORES * D, NSP], f32, addr_space="Shared", tag="ag_out"
                )
                nc.sync.dma_start(ag_in[:], h1T[:])
                nc.gpsimd.collective_compute(
                    "AllGather",
                    mybir.AluOpType.bypass,
                    replica_groups=[list(range(NCORES))],
                    ins=[ag_in.opt()],
                    outs=[ag_out.opt()],
                )
                for c in range(NCORES):
                    nc.sync.dma_start(
                        src[:, c * NSP : (c + 1) * NSP],
                        ag_out[c * D : (c + 1) * D, :],
                    )
                layer(src[:], NGTOT, 1, h2T)
            nc.sync.dma_start(out1[:], h1T[:])
            nc.sync.dma_start(out2[:], h2T[:])
    nc.compile()
    return nc


def _get_compiled(repeat: int = 1):
    if repeat not in _COMPILED:
        _COMPILED[repeat] = _build(repeat)
    return _COMPILED[repeat]


def _idx_layout(ix: np.ndarray) -> np.ndarray:
    """[K, NSP] neighbor ids -> ap_gather SBUF idx layout [128, NCALL, IDXC].

    Per call m the gather order is i = n_off*32 + k over CPC chunks; each Q7
    core reads idx i from (partition i%16, col i//16) of its own group, so
    the 16-wrapped layout is replicated across the 8 groups.
    """
    L = ix.T.reshape(NCALL, GIDX)  # [m, i]
    t16 = L.reshape(NCALL, IDXC, 16)  # [m, s, p16]
    return np.tile(t16.transpose(2, 0, 1), (8, 1, 1)).astype(np.int16)


def _prep_inputs(adjacency, graph, W, b):
    adj = np.asarray(adjacency).astype(np.int64)  # [K, N]
    graph = np.asarray(graph, dtype=np.float32)  # [1, N, D]
    W = np.asarray(W, dtype=np.float32)  # [3, D, D]
    b = np.asarray(b, dtype=np.float32)  # [3, D]

    hsrc0T = np.ascontiguousarray(graph[0].T).astype(np.float32)  # [D, N]
    # W'[d, t, e] = W[t+1][d, e] / K  (mean folded into the weights)
    w_host = np.ascontiguousarray(
        np.stack([W[1], W[2]]).transpose(1, 0, 2) / K
    ).astype(NP_BF16)  # [128(D_in), 2, D_out]
    # b replicated along the free (node) dim; partition dim is the out feature
    b_host = np.ascontiguousarray(
        np.broadcast_to(b[1:3].T[:, :, None], (D, 2, GNOD))
    ).astype(np.float32)  # [128(e), 2, 256(nodes)]

    jj = np.minimum(np.arange(NSP), NS - 1)  # pad nodes clamp to a real node
    in_maps = []
    for c in range(NCORES):
        ga = adj[:, NS * c + jj]  # [K, NSP] global neighbor ids
        idx1 = _idx_layout(ga)
        idx2 = _idx_layout((ga // NS) * NSP + (ga % NS))  # AG padded layout
        in_maps.append(
            {
                "hsrc0T": hsrc0T,
                "idx1": idx1,
                "idx2": idx2,
                "wmat": w_host,
                "brep": b_host,
            }
        )
    return in_maps


def kernel(adjacency, graph, W, b):
    graph = np.asarray(graph, dtype=np.float32)
    in_maps = _prep_inputs(adjacency, graph, W, b)
    nc = _get_compiled(repeat=1)
    res = run_bass_kernel_spmd(nc, in_maps, core_ids=list(range(NCORES)), trace=False)
    h1 = np.concatenate(
        [np.asarray(res.results[c]["out1T"]).T[:NS] for c in range(NCORES)], axis=0
    )
    h2 = np.concatenate(
        [np.asarray(res.results[c]["out2T"]).T[:NS] for c in range(NCORES)], axis=0
    )
    out = np.stack([graph[0], h1, h2], axis=0)[None]  # [1, 3, N, D]
    return out.astype(np.float32)
